# revision 60
# baseline (speedup 1.0000x reference)
"""Trainium2 Bass kernel for nn_Attn2d (3x3 local window attention, 8 heads).

Sharding: 8 cores = (batch 4) x (H halves 2). Each core gets a halo-extended
slice of x (34 rows incl 1-row halo each side, zero-filled outside the image),
computes the 1x1 conv projection + windowed attention for its 32 own rows.

v2 design (cost-model driven):
- PE does only the irreducible matmul work: projection (f32r, bitcast views
  of the raw f32 DMA tiles - no copies), the 9x2 channel reductions into
  logits, pos / Z / recip-broadcast, and the 9x2 identity accumulations of
  the AV products. Biases fold exactly into the pos matrix (b==0 in
  practice; a general-b path adds ones-row matmuls + edge masks).
- attn head->channel expansion is done by SBUF->SBUF broadcast DMAs
  (source AP replicates each attn row 32x via a stride-0 dim), issued from
  whichever engine has slack - this removes all expand matmuls from PE.
- logits rows are ordered dl-major (row = dl*8 + head) so the expand DMA
  reads contiguous partitions.
- q/k/v are stored t-merged [128, 2, px] bf16 so each elementwise product
  handles both channel halves in one op (DVE 2x bf16 mode / Pool flat).
- PE is pre-warmed with dummy matmuls during the input-DMA head so real
  matmuls run at full clock; chunks are software-pipelined
  (logits(ci+1) on PE overlaps AV-products(ci) on DVE/Pool).
"""
import numpy as np

import concourse.mybir as mybir
import concourse.tile as tile
from concourse import bacc

F32 = mybir.dt.float32
F32R = mybir.dt.float32r
BF16 = mybir.dt.bfloat16
AF = mybir.ActivationFunctionType

B, CIN, H, W = 4, 256, 64, 64
QK = 256
OUT = 256
NH = 8
KW = 3
D = QK // NH          # 32
SCALE = float(D) ** (-0.25)
NCORES = 8

HOWN = H // 2         # 32 own rows per core
HS = HOWN + 2         # 34 rows incl halo
WP = W + 4            # 68 padded width (interior cols 2..65)
C0 = 2
PXU = HS * W          # 2176 projection pixels
OWNPX = HOWN * W      # 2048
NKK = KW * KW         # 9
NL = NH * NKK         # 72

CHUNK = 512           # 8 rows per attention chunk
NCHUNK = OWNPX // CHUNK
PXC = [448, 448, 448, 448, 384]           # proj px chunks (7,7,7,7,6 rows)
PXO = [0, 448, 896, 1344, 1792]


def _build_nc(has_bias: bool):
    nc = bacc.Bacc()

    xin = nc.declare_dram_parameter("x", [CIN, PXU], F32, isOutput=False)
    wtd = nc.declare_dram_parameter("wt", [CIN, 3 * QK], F32, isOutput=False)
    posd = nc.declare_dram_parameter("posm", [CIN, NL], BF16, isOutput=False)
    redd = nc.declare_dram_parameter("redm", [CIN, NKK * NL], BF16, isOutput=False)
    sum9d = nc.declare_dram_parameter("sum9", [NL, NH], BF16, isOutput=False)
    e8d = nc.declare_dram_parameter("e8", [NH, NL], BF16, isOutput=False)
    expd = nc.declare_dram_parameter("expm", [NL, 2 * NKK * 128], BF16, isOutput=False)
    identd = nc.declare_dram_parameter("ident", [128, 128], BF16, isOutput=False)
    if has_bias:
        biasd = nc.declare_dram_parameter("bias", [1, 3 * QK], F32, isOutput=False)
        edged = nc.declare_dram_parameter("edge", [128, 2], F32, isOutput=False)
    outd = nc.declare_dram_parameter("o", [OUT, OWNPX], F32, isOutput=True)

    with tile.TileContext(nc) as tc:
        with (
            tc.tile_pool(name="const", bufs=1) as constp,
            tc.tile_pool(name="data", bufs=1) as datap,
            tc.tile_pool(name="work", bufs=4) as workp,
            tc.tile_pool(name="psp", bufs=4, space="PSUM") as psp,   # proj+expand
            tc.tile_pool(name="psl", bufs=1, space="PSUM") as psl,   # logits
            tc.tile_pool(name="psz", bufs=1, space="PSUM") as psz,   # Z + bcast
            tc.tile_pool(name="pso", bufs=1, space="PSUM") as pso,   # AV out
        ):
            # ---- input DMAs: wt on SP/ACT (long poles), x quarters on all
            #      three DMA-capable engines, earliest chunks first ----
            x_t = [datap.tile([128, PXU], F32, tag=f"x{t}", name=f"x{t}")
                   for t in range(2)]
            wt_t = [datap.tile([128, 3 * QK], F32, tag=f"wt{t}", name=f"wt{t}")
                    for t in range(2)]
            # x segment grid aligned to the proj px chunks: q0 = 448 (whole
            # first px chunk), then 288-wide segments. Critical-path first:
            # x q0 halves and wt chunk0 each on their own engine.
            XSEG = [(0, 448), (448, 288), (736, 288), (1024, 288),
                    (1312, 288), (1600, 288), (1888, 288)]
            # ACT's queue starts with a 1.3us LoadActFuncSet, so critical
            # DMAs go on SP/Pool only (DMA data lands at busy_end + ~1.7us)
            nc.gpsimd.dma_start(x_t[0][:, 0:448], xin[0:128, 0:448])
            nc.sync.dma_start(x_t[1][:, 0:448], xin[128:256, 0:448])
            nc.gpsimd.dma_start(wt_t[1][:, 0:256], wtd[128:256, 0:256])
            nc.sync.dma_start(wt_t[0][:, 0:256], wtd[0:128, 0:256])
            for mi in range(1, 3):
                nc.sync.dma_start(wt_t[0][:, mi * 256:(mi + 1) * 256],
                                  wtd[0:128, mi * 256:(mi + 1) * 256])
                nc.scalar.dma_start(wt_t[1][:, mi * 256:(mi + 1) * 256],
                                    wtd[128:256, mi * 256:(mi + 1) * 256])
            XORD = [(1, 0, nc.gpsimd), (1, 1, nc.gpsimd),
                    (2, 0, nc.sync), (2, 1, nc.scalar),
                    (3, 0, nc.sync), (3, 1, nc.scalar),
                    (4, 0, nc.gpsimd), (4, 1, nc.gpsimd),
                    (5, 0, nc.sync), (5, 1, nc.scalar),
                    (6, 0, nc.gpsimd), (6, 1, nc.sync)]
            for qi, t, eng in XORD:
                o0, w = XSEG[qi]
                eng.dma_start(x_t[t][:, o0:o0 + w],
                              xin[t * 128:(t + 1) * 128, o0:o0 + w])
            if has_bias:
                bias_t = constp.tile([1, 3 * QK], F32, tag="bias", name="bias")
                nc.sync.dma_start(bias_t[:], biasd[:])
                edge_t = constp.tile([128, 2], F32, tag="edge", name="edge")
                nc.sync.dma_start(edge_t[:], edged[:])
                ones_t = constp.tile([1, max(PXC)], F32, tag="ones", name="ones")
                nc.gpsimd.memset(ones_t[:], 1.0)

            # ---- q/k/v storage: t-merged bf16; k/v width-padded with halo ----
            q_b = datap.tile([128, 2, PXU], BF16, tag="qb", name="qb")
            k_b = datap.tile([128, 2, HS * WP], BF16, tag="kb", name="kb")
            v_b = datap.tile([128, 2, HS * WP], BF16, tag="vb", name="vb")
            for tl in (k_b, v_b):
                fv = tl[:].bitcast(F32).rearrange("p t (r c) -> p t r c",
                                                  c=WP // 2)
                nc.gpsimd.memset(fv[:, :, :, 0:1], 0.0)
                nc.gpsimd.memset(fv[:, :, :, WP // 2 - 1:WP // 2], 0.0)

            # ---- projection: psum -> writebacks (plain casts; bias via
            #      ones-row matmul only when has_bias) ----
            # f32r operands must be rounded by a compute op (BIR verifier)
            x_rt = [datap.tile([128, PXU], F32R, tag=f"xr{t}", name=f"xr{t}")
                    for t in range(2)]
            wt_rt = [datap.tile([128, 3 * QK], F32R, tag=f"wtr{t}", name=f"wtr{t}")
                     for t in range(2)]
            # fine-grained rounding copies, critical-path first: wt chunk 0
            # and x q0/q1 before the later wt chunks
            def wt_copy(mi, t, eng):
                eng.tensor_copy(wt_rt[t][:, mi * 256:(mi + 1) * 256],
                                wt_t[t][:, mi * 256:(mi + 1) * 256])

            def x_copy(qi, t, eng):
                o0, w = XSEG[qi]
                eng.tensor_copy(x_rt[t][:, o0:o0 + w], x_t[t][:, o0:o0 + w])

            # first-matmul inputs split DVE/Pool so all four land ~3.1us
            x_copy(0, 0, nc.vector)
            x_copy(0, 1, nc.gpsimd)
            wt_copy(0, 0, nc.vector)
            wt_copy(0, 1, nc.gpsimd)
            x_copy(1, 0, nc.vector)
            x_copy(1, 1, nc.gpsimd)
            wt_copy(1, 0, nc.vector)
            wt_copy(1, 1, nc.gpsimd)
            wt_copy(2, 0, nc.gpsimd)
            wt_copy(2, 1, nc.vector)
            for qi in range(2, 7):
                for t in range(2):
                    eng = nc.vector if (qi + t) % 2 == 0 else nc.gpsimd
                    x_copy(qi, t, eng)
            x_r = [x_rt[t][:] for t in range(2)]
            wt_r = [wt_rt[t][:] for t in range(2)]

            # attention constants: emitted after the proj-critical copies,
            # issued from SP/ACT so Pool's queue stays clear
            pos_r = [constp.tile([128, NL], BF16, tag=f"pos{t}", name=f"pos{t}")
                     for t in range(2)]
            red_r = [constp.tile([128, NKK * NL], BF16, tag=f"red{t}", name=f"red{t}")
                     for t in range(2)]
            for t in range(2):
                nc.sync.dma_start(pos_r[t][:], posd[t * 128:(t + 1) * 128, :])
                nc.scalar.dma_start(red_r[t][:], redd[t * 128:(t + 1) * 128, :])
            sum9_r = constp.tile([NL, NH], BF16, tag="sum9", name="sum9")
            nc.sync.dma_start(sum9_r[:], sum9d[:])
            e8_r = constp.tile([NH, NL], BF16, tag="e8", name="e8")
            nc.sync.dma_start(e8_r[:], e8d[:])
            exp_r = constp.tile([NL, 2 * NKK * 128], BF16, tag="expm", name="expm")
            nc.scalar.dma_start(exp_r[:], expd[:])
            ident_r = constp.tile([128, 128], BF16, tag="ident", name="ident")
            nc.sync.dma_start(ident_r[:], identd[:])

            # gpsimd cannot access PSUM: writebacks on ACT/DVE only
            wb_engs = [nc.scalar, nc.scalar, nc.vector, nc.scalar, nc.vector]

            def pad_view(tl, t, r0, nr, c0, cw=W):
                v = tl[:].rearrange("p t (r c) -> p t r c", c=WP)
                return v[:, t, r0:r0 + nr, c0:c0 + cw]

            wb_i = 0
            for ci in range(5):
                cw, co = PXC[ci], PXO[ci]
                r0, nr = co // W, cw // W
                for m in range(6):
                    grp, t = m // 2, m % 2
                    pp = psp.tile([128, CHUNK], F32, tag="pp", name="pp")
                    for kt in range(2):
                        nc.tensor.matmul(
                            pp[:, :cw],
                            wt_r[kt][:, m * 128:(m + 1) * 128],
                            x_r[kt][:, co:co + cw],
                            start=(kt == 0), stop=(not has_bias and kt == 1),
                            skip_group_check=True,
                        )
                    if has_bias:
                        nc.tensor.matmul(
                            pp[:, :cw], bias_t[:, m * 128:(m + 1) * 128],
                            ones_t[:, :cw],
                            start=False, stop=True, skip_group_check=True,
                        )
                    if grp == 0:
                        ov = q_b[:, t, co:co + cw].rearrange(
                            "p (r c) -> p r c", c=W)
                    else:
                        ov = pad_view(k_b if grp == 1 else v_b, t, r0, nr, C0)
                    eng = wb_engs[wb_i % len(wb_engs)]
                    wb_i += 1
                    if eng is nc.scalar:
                        nc.scalar.copy(ov, pp[:, :cw].rearrange(
                            "p (r c) -> p r c", c=W))
                    else:
                        eng.tensor_copy(ov, pp[:, :cw].rearrange(
                            "p (r c) -> p r c", c=W))

            if has_bias:
                # zero k/v halo rows that fall outside the image
                for tl in (k_b, v_b):
                    pv = tl[:].rearrange("p t (r c) -> p t r c", c=WP)
                    nc.gpsimd.tensor_scalar_mul(pv[:, :, 0, :], pv[:, :, 0, :],
                                                edge_t[:, 0:1])
                    nc.gpsimd.tensor_scalar_mul(pv[:, :, HS - 1, :],
                                                pv[:, :, HS - 1, :],
                                                edge_t[:, 1:2])

            # ---- attention chunks ----
            def qview(ci):
                return q_b[:].rearrange("p t (r c) -> p t r c", c=W)[
                    :, :, 1 + 8 * ci:9 + 8 * ci, :]

            def kv_view(tl, ci, di, dj):
                return tl[:].rearrange("p t (r c) -> p t r c", c=WP)[
                    :, :, 8 * ci + di:8 * ci + di + 8, dj + 1:dj + 1 + W]

            # product engine assignment per dl: reduce phase / AV phase
            RED_ENG = [0, 1, 0, 0, 1, 0, 0, 1, 0]   # 0=DVE (6), 1=Pool (3)
            # AV: 0 = DVE direct from psum; 1 = ACT cast to SBUF + Pool mul
            AV_ENG = [(1, 0), (0, 1), (0, 1), (1, 0), (0, 1), (0, 1),
                      (1, 0), (1, 0), (0, 1)]       # DVE 10, ACT+Pool 8

            def emit_logits(ci):
                prs = []
                for dl in range(NKK):
                    di, dj = dl // KW, dl % KW
                    pr = workp.tile([128, 2, CHUNK], BF16, tag="pr",
                                    name=f"pr{ci}_{dl}", bufs=6)
                    eng = nc.vector if RED_ENG[dl] == 0 else nc.gpsimd
                    eng.tensor_mul(
                        pr[:].rearrange("p t (r c) -> p t r c", c=W),
                        qview(ci), kv_view(k_b, ci, di, dj))
                    prs.append(pr)
                pl = psl.tile([NL, CHUNK], F32, tag="pl", name=f"pl{ci}", bufs=1)
                for t in range(2):
                    nc.tensor.matmul(pl[:], pos_r[t][:],
                                     q_b[:, t, 64 + CHUNK * ci:64 + CHUNK * (ci + 1)],
                                     start=(t == 0), stop=False,
                                     skip_group_check=True)
                for dl in range(NKK):
                    for t in range(2):
                        nc.tensor.matmul(
                            pl[:], red_r[t][:, dl * NL:(dl + 1) * NL],
                            prs[dl][:, t, :],
                            start=False, stop=(dl == NKK - 1 and t == 1),
                            skip_group_check=True)
                e_t = workp.tile([NL, CHUNK], BF16, tag="e", name=f"e{ci}", bufs=2)
                nc.scalar.activation(e_t[:], pl[:], AF.Exp)
                zz = psz.tile([NL, CHUNK], F32, tag="zz", name=f"zz{ci}", bufs=1)
                nc.tensor.matmul(zz[:][64:72], sum9_r[:], e_t[:],
                                 start=True, stop=True, skip_group_check=True)
                rz = workp.tile([NH, CHUNK], BF16, tag="rz", name=f"rz{ci}", bufs=2)
                with nc.allow_low_precision(reason="bf16 softmax denominators"):
                    nc.vector.reciprocal(rz[:], zz[:][64:72])
                nc.tensor.matmul(zz[:][0:NL], e8_r[:], rz[:],
                                 start=True, stop=True, skip_group_check=True)
                attn = workp.tile([NL, CHUNK], BF16, tag="attn",
                                  name=f"attn{ci}", bufs=2)
                nc.vector.tensor_mul(attn[:], e_t[:], zz[:][0:NL])
                return attn

            def emit_av(ci, attn):
                # per dl: 2 expand matmuls (psum, pp slots) -> 2 products
                # -> 1 flat ident matmul accumulating both halves into po
                po = pso.tile([128, 2, CHUNK], F32, tag="po", name=f"po{ci}",
                              bufs=1)
                p2s = [None] * NKK

                def emit_exp_prod(dl):
                    di, dj = dl // KW, dl % KW
                    p2 = workp.tile([128, 2, CHUNK], BF16, tag="p2",
                                    name=f"p2{ci}_{dl}", bufs=5)
                    for t in range(2):
                        pe = psp.tile([128, CHUNK], F32, tag="pp",
                                      name=f"ax{ci}_{dl}_{t}")
                        nc.tensor.matmul(
                            pe[:], exp_r[:, (dl * 2 + t) * 128:(dl * 2 + t + 1) * 128],
                            attn[:], start=True, stop=True,
                            skip_group_check=True)
                        if AV_ENG[dl][t] == 0:
                            # DVE multiplies straight from psum
                            nc.vector.tensor_mul(
                                p2[:, t, :].rearrange("p (r c) -> p r c", c=W),
                                pe[:].rearrange("p (r c) -> p r c", c=W),
                                kv_view(v_b, ci, di, dj)[:, t])
                        else:
                            # gpsimd can't read psum: ACT casts, Pool multiplies
                            axs = workp.tile([128, CHUNK], BF16, tag="axs",
                                             name=f"axs{ci}_{dl}_{t}", bufs=4)
                            nc.scalar.copy(axs[:], pe[:])
                            nc.gpsimd.tensor_mul(
                                p2[:, t, :].rearrange("p (r c) -> p r c", c=W),
                                axs[:].rearrange("p (r c) -> p r c", c=W),
                                kv_view(v_b, ci, di, dj)[:, t])
                    p2s[dl] = p2

                def emit_ident(dl, t):
                    nc.tensor.matmul(
                        po[:, t, :], ident_r[:], p2s[dl][:, t, :],
                        start=(dl == 0), stop=(dl == NKK - 1),
                        skip_group_check=True)

                # t0 chain runs one dl ahead of t1 so po[t0] closes first
                # and its drain overlaps the final t1 idents
                emit_exp_prod(0)
                emit_exp_prod(1)
                emit_ident(0, 0)
                for dl in range(2, NKK):
                    emit_exp_prod(dl)
                    emit_ident(dl - 1, 0)
                    emit_ident(dl - 2, 1)
                emit_ident(NKK - 1, 0)
                ob = workp.tile([128, 2, CHUNK], F32, tag="ob",
                                name=f"ob{ci}", bufs=2)
                ovw = outd[:].rearrange("(t c) px -> c t px", t=2)
                nc.scalar.copy(ob[:, 0, :], po[:, 0, :])
                nc.sync.dma_start(ovw[:, 0, ci * CHUNK:(ci + 1) * CHUNK],
                                  ob[:, 0, :])
                emit_ident(NKK - 2, 1)
                emit_ident(NKK - 1, 1)
                if ci == NCHUNK - 1:
                    # tail: drain t1 in two half-px pieces so the first DMA
                    # issues while the second half is still being copied
                    hc = CHUNK // 2
                    for h in range(2):
                        nc.scalar.copy(ob[:, 1, h * hc:(h + 1) * hc],
                                       po[:, 1, h * hc:(h + 1) * hc])
                        nc.sync.dma_start(
                            ovw[:, 1, ci * CHUNK + h * hc:ci * CHUNK + (h + 1) * hc],
                            ob[:, 1, h * hc:(h + 1) * hc])
                else:
                    nc.scalar.copy(ob[:, 1, :], po[:, 1, :])
                    nc.sync.dma_start(ovw[:, 1, ci * CHUNK:(ci + 1) * CHUNK],
                                      ob[:, 1, :])

            # software pipeline: logits(ci+1) on PE ahead of AV(ci)
            attn_prev = emit_logits(0)
            for ci in range(1, NCHUNK):
                attn_c = emit_logits(ci)
                emit_av(ci - 1, attn_prev)
                attn_prev = attn_c
            emit_av(NCHUNK - 1, attn_prev)

    nc.finalize()
    return nc


_CACHE = {}


def _host_consts(w_proj, b_proj, pos_feats):
    wT = np.ascontiguousarray(w_proj.T).astype(np.float32).copy()   # [256, 768]
    wT[:, :2 * QK] *= SCALE

    import ml_dtypes
    bf = ml_dtypes.bfloat16

    heads = np.arange(CIN) // D
    posm = np.zeros((CIN, NL), np.float32)
    for g in range(CIN):
        n = heads[g]
        for dl in range(NKK):
            posm[g, dl * NH + n] = pos_feats[g, dl]

    redm = np.zeros((CIN, NKK * NL), np.float32)
    for t in range(2):
        for c in range(128):
            n = heads[t * 128 + c]
            for dl in range(NKK):
                redm[t * 128 + c, dl * NL + dl * NH + n] = 1.0
    # NOTE: redm rows are global channels; tile t uses rows t*128..t*128+127

    sum9 = np.zeros((NL, NH), np.float32)
    e8 = np.zeros((NH, NL), np.float32)
    for n in range(NH):
        for dl in range(NKK):
            sum9[dl * NH + n, n] = 1.0
            e8[n, dl * NH + n] = 1.0

    expm = np.zeros((NL, 2 * NKK * 128), np.float32)
    for dl in range(NKK):
        for t in range(2):
            for c in range(128):
                expm[dl * NH + t * 4 + c // 32, (dl * 2 + t) * 128 + c] = 1.0

    ident = np.eye(128, dtype=np.float32)

    b = np.asarray(b_proj, np.float32).copy()
    b[:2 * QK] *= SCALE
    bias = np.ascontiguousarray(b.reshape(1, 3 * QK))

    return (wT, posm.astype(bf), redm.astype(bf), sum9.astype(bf),
            e8.astype(bf), expm.astype(bf), ident.astype(bf), bias)


def make_in_maps(x, w_proj, b_proj, pos_feats):
    x = np.asarray(x, np.float32)
    has_bias = bool(np.any(np.asarray(b_proj)))
    wT, posm, redm, sum9, e8, expm, ident, bias = _host_consts(
        np.asarray(w_proj, np.float32),
        np.asarray(b_proj, np.float32),
        np.asarray(pos_feats, np.float32),
    )
    in_maps = []
    for s in range(NCORES):
        b_i, half = s // 2, s % 2
        xs = np.zeros((CIN, HS, W), np.float32)
        h0 = half * HOWN - 1
        lo, hi = max(h0, 0), min(h0 + HS, H)
        xs[:, lo - h0:hi - h0, :] = x[b_i, :, lo:hi, :]
        m = {
            "x": np.ascontiguousarray(xs.reshape(CIN, PXU)),
            "wt": wT, "posm": posm, "redm": redm,
            "sum9": sum9, "e8": e8, "expm": expm, "ident": ident,
        }
        if has_bias:
            edge = np.ones((128, 2), np.float32)
            if half == 0:
                edge[:, 0] = 0.0
            if half == 1:
                edge[:, 1] = 0.0
            m["bias"] = bias
            m["edge"] = edge
        in_maps.append(m)
    return in_maps, has_bias


def kernel(x, w_proj, b_proj, pos_feats):
    from concourse.bass_utils import run_bass_kernel_spmd

    in_maps, has_bias = make_in_maps(x, w_proj, b_proj, pos_feats)
    key = ("nc", has_bias)
    if key not in _CACHE:
        _CACHE[key] = _build_nc(has_bias)
        _CACHE["nc"] = _CACHE[key]
    nc = _CACHE[key]
    res = run_bass_kernel_spmd(nc, in_maps, list(range(NCORES)))
    out = np.zeros((B, OUT, H, W), np.float32)
    for s in range(NCORES):
        b_i, half = s // 2, s % 2
        out[b_i, :, half * HOWN:(half + 1) * HOWN, :] = (
            res.results[s]["o"].reshape(OUT, HOWN, W)
        )
    return out


# revision 61
# speedup vs baseline: 1.0076x; 1.0076x over previous
"""Trainium2 Bass kernel for nn_Attn2d (3x3 local window attention, 8 heads).

Sharding: 8 cores = (batch 4) x (H halves 2). Each core gets a halo-extended
slice of x (34 rows incl 1-row halo each side, zero-filled outside the image),
computes the 1x1 conv projection + windowed attention for its 32 own rows.

v2 design (cost-model driven):
- PE does only the irreducible matmul work: projection (f32r, bitcast views
  of the raw f32 DMA tiles - no copies), the 9x2 channel reductions into
  logits, pos / Z / recip-broadcast, and the 9x2 identity accumulations of
  the AV products. Biases fold exactly into the pos matrix (b==0 in
  practice; a general-b path adds ones-row matmuls + edge masks).
- attn head->channel expansion is done by SBUF->SBUF broadcast DMAs
  (source AP replicates each attn row 32x via a stride-0 dim), issued from
  whichever engine has slack - this removes all expand matmuls from PE.
- logits rows are ordered dl-major (row = dl*8 + head) so the expand DMA
  reads contiguous partitions.
- q/k/v are stored t-merged [128, 2, px] bf16 so each elementwise product
  handles both channel halves in one op (DVE 2x bf16 mode / Pool flat).
- PE is pre-warmed with dummy matmuls during the input-DMA head so real
  matmuls run at full clock; chunks are software-pipelined
  (logits(ci+1) on PE overlaps AV-products(ci) on DVE/Pool).
"""
import numpy as np

import concourse.mybir as mybir
import concourse.tile as tile
from concourse import bacc

F32 = mybir.dt.float32
F32R = mybir.dt.float32r
BF16 = mybir.dt.bfloat16
AF = mybir.ActivationFunctionType

B, CIN, H, W = 4, 256, 64, 64
QK = 256
OUT = 256
NH = 8
KW = 3
D = QK // NH          # 32
SCALE = float(D) ** (-0.25)
NCORES = 8

HOWN = H // 2         # 32 own rows per core
HS = HOWN + 2         # 34 rows incl halo
WP = W + 4            # 68 padded width (interior cols 2..65)
C0 = 2
PXU = HS * W          # 2176 projection pixels
OWNPX = HOWN * W      # 2048
NKK = KW * KW         # 9
NL = NH * NKK         # 72

CHUNK = 512           # 8 rows per attention chunk
NCHUNK = OWNPX // CHUNK
PXC = [448, 448, 448, 448, 384]           # proj px chunks (7,7,7,7,6 rows)
PXO = [0, 448, 896, 1344, 1792]


def _build_nc(has_bias: bool):
    nc = bacc.Bacc()

    xin = nc.declare_dram_parameter("x", [CIN, PXU], F32, isOutput=False)
    wtd = nc.declare_dram_parameter("wt", [CIN, 3 * QK], F32, isOutput=False)
    posd = nc.declare_dram_parameter("posm", [CIN, NL], BF16, isOutput=False)
    redd = nc.declare_dram_parameter("redm", [CIN, NKK * NL], BF16, isOutput=False)
    sum9d = nc.declare_dram_parameter("sum9", [NL, NH], BF16, isOutput=False)
    e8d = nc.declare_dram_parameter("e8", [NH, NL], BF16, isOutput=False)
    expd = nc.declare_dram_parameter("expm", [NL, 2 * NKK * 128], BF16, isOutput=False)
    identd = nc.declare_dram_parameter("ident", [128, 128], BF16, isOutput=False)
    if has_bias:
        biasd = nc.declare_dram_parameter("bias", [1, 3 * QK], F32, isOutput=False)
        edged = nc.declare_dram_parameter("edge", [128, 2], F32, isOutput=False)
    outd = nc.declare_dram_parameter("o", [OUT, OWNPX], F32, isOutput=True)

    with tile.TileContext(nc) as tc:
        with (
            tc.tile_pool(name="const", bufs=1) as constp,
            tc.tile_pool(name="data", bufs=1) as datap,
            tc.tile_pool(name="work", bufs=4) as workp,
            tc.tile_pool(name="psp", bufs=4, space="PSUM") as psp,   # proj+expand
            tc.tile_pool(name="psl", bufs=1, space="PSUM") as psl,   # logits
            tc.tile_pool(name="psz", bufs=1, space="PSUM") as psz,   # Z + bcast
            tc.tile_pool(name="pso", bufs=1, space="PSUM") as pso,   # AV out
        ):
            # ---- input DMAs: wt on SP/ACT (long poles), x quarters on all
            #      three DMA-capable engines, earliest chunks first ----
            x_t = [datap.tile([128, PXU], F32, tag=f"x{t}", name=f"x{t}")
                   for t in range(2)]
            wt_t = [datap.tile([128, 3 * QK], F32, tag=f"wt{t}", name=f"wt{t}")
                    for t in range(2)]
            # x segment grid aligned to the proj px chunks: q0 = 448 (whole
            # first px chunk), then 288-wide segments. Critical-path first:
            # x q0 halves and wt chunk0 each on their own engine.
            XSEG = [(0, 448), (448, 288), (736, 288), (1024, 288),
                    (1312, 288), (1600, 288), (1888, 288)]
            # ACT's queue starts with a 1.3us LoadActFuncSet, so critical
            # DMAs go on SP/Pool only (DMA data lands at busy_end + ~1.7us)
            nc.gpsimd.dma_start(x_t[0][:, 0:448], xin[0:128, 0:448])
            nc.sync.dma_start(x_t[1][:, 0:448], xin[128:256, 0:448])
            nc.gpsimd.dma_start(wt_t[1][:, 0:256], wtd[128:256, 0:256])
            nc.sync.dma_start(wt_t[0][:, 0:256], wtd[0:128, 0:256])
            for mi in range(1, 3):
                nc.sync.dma_start(wt_t[0][:, mi * 256:(mi + 1) * 256],
                                  wtd[0:128, mi * 256:(mi + 1) * 256])
                nc.scalar.dma_start(wt_t[1][:, mi * 256:(mi + 1) * 256],
                                    wtd[128:256, mi * 256:(mi + 1) * 256])
            XORD = [(1, 0, nc.gpsimd), (1, 1, nc.gpsimd),
                    (2, 0, nc.sync), (2, 1, nc.scalar),
                    (3, 0, nc.sync), (3, 1, nc.scalar),
                    (4, 0, nc.gpsimd), (4, 1, nc.gpsimd),
                    (5, 0, nc.sync), (5, 1, nc.scalar),
                    (6, 0, nc.gpsimd), (6, 1, nc.sync)]
            for qi, t, eng in XORD:
                o0, w = XSEG[qi]
                eng.dma_start(x_t[t][:, o0:o0 + w],
                              xin[t * 128:(t + 1) * 128, o0:o0 + w])
            if has_bias:
                bias_t = constp.tile([1, 3 * QK], F32, tag="bias", name="bias")
                nc.sync.dma_start(bias_t[:], biasd[:])
                edge_t = constp.tile([128, 2], F32, tag="edge", name="edge")
                nc.sync.dma_start(edge_t[:], edged[:])
                ones_t = constp.tile([1, max(PXC)], F32, tag="ones", name="ones")
                nc.gpsimd.memset(ones_t[:], 1.0)

            # ---- q/k/v storage: t-merged bf16; k/v width-padded with halo ----
            q_b = datap.tile([128, 2, PXU], BF16, tag="qb", name="qb")
            k_b = datap.tile([128, 2, HS * WP], BF16, tag="kb", name="kb")
            v_b = datap.tile([128, 2, HS * WP], BF16, tag="vb", name="vb")
            for tl in (k_b, v_b):
                fv = tl[:].bitcast(F32).rearrange("p t (r c) -> p t r c",
                                                  c=WP // 2)
                nc.gpsimd.memset(fv[:, :, :, 0:1], 0.0)
                nc.gpsimd.memset(fv[:, :, :, WP // 2 - 1:WP // 2], 0.0)

            # ---- projection: psum -> writebacks (plain casts; bias via
            #      ones-row matmul only when has_bias) ----
            # f32r operands must be rounded by a compute op (BIR verifier)
            x_rt = [datap.tile([128, PXU], F32R, tag=f"xr{t}", name=f"xr{t}")
                    for t in range(2)]
            wt_rt = [datap.tile([128, 3 * QK], F32R, tag=f"wtr{t}", name=f"wtr{t}")
                     for t in range(2)]
            # fine-grained rounding copies, critical-path first: wt chunk 0
            # and x q0/q1 before the later wt chunks
            def wt_copy(mi, t, eng):
                eng.tensor_copy(wt_rt[t][:, mi * 256:(mi + 1) * 256],
                                wt_t[t][:, mi * 256:(mi + 1) * 256])

            def x_copy(qi, t, eng):
                o0, w = XSEG[qi]
                eng.tensor_copy(x_rt[t][:, o0:o0 + w], x_t[t][:, o0:o0 + w])

            # first-matmul inputs split DVE/Pool so all four land ~3.1us
            x_copy(0, 0, nc.vector)
            x_copy(0, 1, nc.gpsimd)
            wt_copy(0, 0, nc.vector)
            wt_copy(0, 1, nc.gpsimd)
            x_copy(1, 0, nc.vector)
            x_copy(1, 1, nc.gpsimd)
            wt_copy(1, 0, nc.vector)
            wt_copy(1, 1, nc.gpsimd)
            wt_copy(2, 0, nc.gpsimd)
            wt_copy(2, 1, nc.vector)
            for qi in range(2, 7):
                for t in range(2):
                    eng = nc.vector if (qi + t) % 2 == 0 else nc.gpsimd
                    x_copy(qi, t, eng)
            x_r = [x_rt[t][:] for t in range(2)]
            wt_r = [wt_rt[t][:] for t in range(2)]

            # attention constants: emitted after the proj-critical copies,
            # issued from SP/ACT so Pool's queue stays clear
            pos_r = [constp.tile([128, NL], BF16, tag=f"pos{t}", name=f"pos{t}")
                     for t in range(2)]
            red_r = [constp.tile([128, NKK * NL], BF16, tag=f"red{t}", name=f"red{t}")
                     for t in range(2)]
            for t in range(2):
                nc.sync.dma_start(pos_r[t][:], posd[t * 128:(t + 1) * 128, :])
                nc.scalar.dma_start(red_r[t][:], redd[t * 128:(t + 1) * 128, :])
            sum9_r = constp.tile([NL, NH], BF16, tag="sum9", name="sum9")
            nc.sync.dma_start(sum9_r[:], sum9d[:])
            e8_r = constp.tile([NH, NL], BF16, tag="e8", name="e8")
            nc.sync.dma_start(e8_r[:], e8d[:])
            exp_r = constp.tile([NL, 2 * NKK * 128], BF16, tag="expm", name="expm")
            nc.scalar.dma_start(exp_r[:], expd[:])
            ident_r = constp.tile([128, 128], BF16, tag="ident", name="ident")
            nc.sync.dma_start(ident_r[:], identd[:])

            # gpsimd cannot access PSUM: writebacks on ACT/DVE only
            wb_engs = [nc.scalar, nc.scalar, nc.vector]

            def pad_view(tl, t, r0, nr, c0, cw=W):
                v = tl[:].rearrange("p t (r c) -> p t r c", c=WP)
                return v[:, t, r0:r0 + nr, c0:c0 + cw]

            wb_i = 0
            for ci in range(5):
                cw, co = PXC[ci], PXO[ci]
                r0, nr = co // W, cw // W
                for m in range(6):
                    grp, t = m // 2, m % 2
                    pp = psp.tile([128, CHUNK], F32, tag="pp", name="pp")
                    for kt in range(2):
                        nc.tensor.matmul(
                            pp[:, :cw],
                            wt_r[kt][:, m * 128:(m + 1) * 128],
                            x_r[kt][:, co:co + cw],
                            start=(kt == 0), stop=(not has_bias and kt == 1),
                            skip_group_check=True,
                        )
                    if has_bias:
                        nc.tensor.matmul(
                            pp[:, :cw], bias_t[:, m * 128:(m + 1) * 128],
                            ones_t[:, :cw],
                            start=False, stop=True, skip_group_check=True,
                        )
                    if grp == 0:
                        ov = q_b[:, t, co:co + cw].rearrange(
                            "p (r c) -> p r c", c=W)
                    else:
                        ov = pad_view(k_b if grp == 1 else v_b, t, r0, nr, C0)
                    eng = wb_engs[wb_i % len(wb_engs)]
                    wb_i += 1
                    if eng is nc.scalar:
                        nc.scalar.copy(ov, pp[:, :cw].rearrange(
                            "p (r c) -> p r c", c=W))
                    else:
                        eng.tensor_copy(ov, pp[:, :cw].rearrange(
                            "p (r c) -> p r c", c=W))

            if has_bias:
                # zero k/v halo rows that fall outside the image
                for tl in (k_b, v_b):
                    pv = tl[:].rearrange("p t (r c) -> p t r c", c=WP)
                    nc.gpsimd.tensor_scalar_mul(pv[:, :, 0, :], pv[:, :, 0, :],
                                                edge_t[:, 0:1])
                    nc.gpsimd.tensor_scalar_mul(pv[:, :, HS - 1, :],
                                                pv[:, :, HS - 1, :],
                                                edge_t[:, 1:2])

            # ---- attention chunks ----
            def qview(ci):
                return q_b[:].rearrange("p t (r c) -> p t r c", c=W)[
                    :, :, 1 + 8 * ci:9 + 8 * ci, :]

            def kv_view(tl, ci, di, dj):
                return tl[:].rearrange("p t (r c) -> p t r c", c=WP)[
                    :, :, 8 * ci + di:8 * ci + di + 8, dj + 1:dj + 1 + W]

            # product engine assignment per dl: reduce phase / AV phase
            RED_ENG = [0, 1, 0, 0, 1, 0, 0, 1, 0]   # 0=DVE (6), 1=Pool (3)
            # AV: 0 = DVE direct from psum; 1 = ACT cast to SBUF + Pool mul
            AV_ENG = [(1, 0), (0, 1), (0, 1), (1, 0), (0, 1), (0, 1),
                      (1, 0), (1, 0), (0, 1)]       # DVE 10, ACT+Pool 8

            def emit_logits(ci):
                prs = []
                for dl in range(NKK):
                    di, dj = dl // KW, dl % KW
                    pr = workp.tile([128, 2, CHUNK], BF16, tag="pr",
                                    name=f"pr{ci}_{dl}", bufs=6)
                    eng = nc.vector if RED_ENG[dl] == 0 else nc.gpsimd
                    eng.tensor_mul(
                        pr[:].rearrange("p t (r c) -> p t r c", c=W),
                        qview(ci), kv_view(k_b, ci, di, dj))
                    prs.append(pr)
                pl = psl.tile([NL, CHUNK], F32, tag="pl", name=f"pl{ci}", bufs=1)
                for t in range(2):
                    nc.tensor.matmul(pl[:], pos_r[t][:],
                                     q_b[:, t, 64 + CHUNK * ci:64 + CHUNK * (ci + 1)],
                                     start=(t == 0), stop=False,
                                     skip_group_check=True)
                for dl in range(NKK):
                    for t in range(2):
                        nc.tensor.matmul(
                            pl[:], red_r[t][:, dl * NL:(dl + 1) * NL],
                            prs[dl][:, t, :],
                            start=False, stop=(dl == NKK - 1 and t == 1),
                            skip_group_check=True)
                e_t = workp.tile([NL, CHUNK], BF16, tag="e", name=f"e{ci}", bufs=2)
                nc.scalar.activation(e_t[:], pl[:], AF.Exp)
                zz = psz.tile([NL, CHUNK], F32, tag="zz", name=f"zz{ci}", bufs=1)
                nc.tensor.matmul(zz[:][64:72], sum9_r[:], e_t[:],
                                 start=True, stop=True, skip_group_check=True)
                rz = workp.tile([NH, CHUNK], BF16, tag="rz", name=f"rz{ci}", bufs=2)
                with nc.allow_low_precision(reason="bf16 softmax denominators"):
                    nc.vector.reciprocal(rz[:], zz[:][64:72])
                nc.tensor.matmul(zz[:][0:NL], e8_r[:], rz[:],
                                 start=True, stop=True, skip_group_check=True)
                attn = workp.tile([NL, CHUNK], BF16, tag="attn",
                                  name=f"attn{ci}", bufs=2)
                nc.vector.tensor_mul(attn[:], e_t[:], zz[:][0:NL])
                return attn

            def emit_av(ci, attn):
                # per dl: 2 expand matmuls (psum, pp slots) -> 2 products
                # -> 1 flat ident matmul accumulating both halves into po
                po = pso.tile([128, 2, CHUNK], F32, tag="po", name=f"po{ci}",
                              bufs=1)
                p2s = [None] * NKK

                def emit_exp_prod(dl):
                    di, dj = dl // KW, dl % KW
                    p2 = workp.tile([128, 2, CHUNK], BF16, tag="p2",
                                    name=f"p2{ci}_{dl}", bufs=5)
                    for t in range(2):
                        pe = psp.tile([128, CHUNK], F32, tag="pp",
                                      name=f"ax{ci}_{dl}_{t}")
                        nc.tensor.matmul(
                            pe[:], exp_r[:, (dl * 2 + t) * 128:(dl * 2 + t + 1) * 128],
                            attn[:], start=True, stop=True,
                            skip_group_check=True)
                        if AV_ENG[dl][t] == 0:
                            # DVE multiplies straight from psum
                            nc.vector.tensor_mul(
                                p2[:, t, :].rearrange("p (r c) -> p r c", c=W),
                                pe[:].rearrange("p (r c) -> p r c", c=W),
                                kv_view(v_b, ci, di, dj)[:, t])
                        else:
                            # gpsimd can't read psum: ACT casts, Pool multiplies
                            axs = workp.tile([128, CHUNK], BF16, tag="axs",
                                             name=f"axs{ci}_{dl}_{t}", bufs=4)
                            nc.scalar.copy(axs[:], pe[:])
                            nc.gpsimd.tensor_mul(
                                p2[:, t, :].rearrange("p (r c) -> p r c", c=W),
                                axs[:].rearrange("p (r c) -> p r c", c=W),
                                kv_view(v_b, ci, di, dj)[:, t])
                    p2s[dl] = p2

                def emit_ident(dl, t):
                    nc.tensor.matmul(
                        po[:, t, :], ident_r[:], p2s[dl][:, t, :],
                        start=(dl == 0), stop=(dl == NKK - 1),
                        skip_group_check=True)

                # t0 chain runs one dl ahead of t1 so po[t0] closes first
                # and its drain overlaps the final t1 idents
                emit_exp_prod(0)
                emit_exp_prod(1)
                emit_ident(0, 0)
                for dl in range(2, NKK):
                    emit_exp_prod(dl)
                    emit_ident(dl - 1, 0)
                    emit_ident(dl - 2, 1)
                emit_ident(NKK - 1, 0)
                ob = workp.tile([128, 2, CHUNK], F32, tag="ob",
                                name=f"ob{ci}", bufs=2)
                ovw = outd[:].rearrange("(t c) px -> c t px", t=2)
                nc.scalar.copy(ob[:, 0, :], po[:, 0, :])
                nc.sync.dma_start(ovw[:, 0, ci * CHUNK:(ci + 1) * CHUNK],
                                  ob[:, 0, :])
                emit_ident(NKK - 2, 1)
                emit_ident(NKK - 1, 1)
                if ci == NCHUNK - 1:
                    # tail: drain t1 in two half-px pieces so the first DMA
                    # issues while the second half is still being copied
                    hc = CHUNK // 2
                    for h in range(2):
                        nc.scalar.copy(ob[:, 1, h * hc:(h + 1) * hc],
                                       po[:, 1, h * hc:(h + 1) * hc])
                        nc.sync.dma_start(
                            ovw[:, 1, ci * CHUNK + h * hc:ci * CHUNK + (h + 1) * hc],
                            ob[:, 1, h * hc:(h + 1) * hc])
                else:
                    nc.scalar.copy(ob[:, 1, :], po[:, 1, :])
                    nc.sync.dma_start(ovw[:, 1, ci * CHUNK:(ci + 1) * CHUNK],
                                      ob[:, 1, :])

            # software pipeline: logits(ci+1) on PE ahead of AV(ci)
            attn_prev = emit_logits(0)
            for ci in range(1, NCHUNK):
                attn_c = emit_logits(ci)
                emit_av(ci - 1, attn_prev)
                attn_prev = attn_c
            emit_av(NCHUNK - 1, attn_prev)

    nc.finalize()
    return nc


_CACHE = {}


def _host_consts(w_proj, b_proj, pos_feats):
    wT = np.ascontiguousarray(w_proj.T).astype(np.float32).copy()   # [256, 768]
    wT[:, :2 * QK] *= SCALE

    import ml_dtypes
    bf = ml_dtypes.bfloat16

    heads = np.arange(CIN) // D
    posm = np.zeros((CIN, NL), np.float32)
    for g in range(CIN):
        n = heads[g]
        for dl in range(NKK):
            posm[g, dl * NH + n] = pos_feats[g, dl]

    redm = np.zeros((CIN, NKK * NL), np.float32)
    for t in range(2):
        for c in range(128):
            n = heads[t * 128 + c]
            for dl in range(NKK):
                redm[t * 128 + c, dl * NL + dl * NH + n] = 1.0
    # NOTE: redm rows are global channels; tile t uses rows t*128..t*128+127

    sum9 = np.zeros((NL, NH), np.float32)
    e8 = np.zeros((NH, NL), np.float32)
    for n in range(NH):
        for dl in range(NKK):
            sum9[dl * NH + n, n] = 1.0
            e8[n, dl * NH + n] = 1.0

    expm = np.zeros((NL, 2 * NKK * 128), np.float32)
    for dl in range(NKK):
        for t in range(2):
            for c in range(128):
                expm[dl * NH + t * 4 + c // 32, (dl * 2 + t) * 128 + c] = 1.0

    ident = np.eye(128, dtype=np.float32)

    b = np.asarray(b_proj, np.float32).copy()
    b[:2 * QK] *= SCALE
    bias = np.ascontiguousarray(b.reshape(1, 3 * QK))

    return (wT, posm.astype(bf), redm.astype(bf), sum9.astype(bf),
            e8.astype(bf), expm.astype(bf), ident.astype(bf), bias)


def make_in_maps(x, w_proj, b_proj, pos_feats):
    x = np.asarray(x, np.float32)
    has_bias = bool(np.any(np.asarray(b_proj)))
    wT, posm, redm, sum9, e8, expm, ident, bias = _host_consts(
        np.asarray(w_proj, np.float32),
        np.asarray(b_proj, np.float32),
        np.asarray(pos_feats, np.float32),
    )
    in_maps = []
    for s in range(NCORES):
        b_i, half = s // 2, s % 2
        xs = np.zeros((CIN, HS, W), np.float32)
        h0 = half * HOWN - 1
        lo, hi = max(h0, 0), min(h0 + HS, H)
        xs[:, lo - h0:hi - h0, :] = x[b_i, :, lo:hi, :]
        m = {
            "x": np.ascontiguousarray(xs.reshape(CIN, PXU)),
            "wt": wT, "posm": posm, "redm": redm,
            "sum9": sum9, "e8": e8, "expm": expm, "ident": ident,
        }
        if has_bias:
            edge = np.ones((128, 2), np.float32)
            if half == 0:
                edge[:, 0] = 0.0
            if half == 1:
                edge[:, 1] = 0.0
            m["bias"] = bias
            m["edge"] = edge
        in_maps.append(m)
    return in_maps, has_bias


def kernel(x, w_proj, b_proj, pos_feats):
    from concourse.bass_utils import run_bass_kernel_spmd

    in_maps, has_bias = make_in_maps(x, w_proj, b_proj, pos_feats)
    key = ("nc", has_bias)
    if key not in _CACHE:
        _CACHE[key] = _build_nc(has_bias)
        _CACHE["nc"] = _CACHE[key]
    nc = _CACHE[key]
    res = run_bass_kernel_spmd(nc, in_maps, list(range(NCORES)))
    out = np.zeros((B, OUT, H, W), np.float32)
    for s in range(NCORES):
        b_i, half = s // 2, s % 2
        out[b_i, :, half * HOWN:(half + 1) * HOWN, :] = (
            res.results[s]["o"].reshape(OUT, HOWN, W)
        )
    return out


# revision 62
# speedup vs baseline: 1.0162x; 1.0085x over previous
"""Trainium2 Bass kernel for nn_Attn2d (3x3 local window attention, 8 heads).

Sharding: 8 cores = (batch 4) x (H halves 2). Each core gets a halo-extended
slice of x (34 rows incl 1-row halo each side, zero-filled outside the image),
computes the 1x1 conv projection + windowed attention for its 32 own rows.

v2 design (cost-model driven):
- PE does only the irreducible matmul work: projection (f32r, bitcast views
  of the raw f32 DMA tiles - no copies), the 9x2 channel reductions into
  logits, pos / Z / recip-broadcast, and the 9x2 identity accumulations of
  the AV products. Biases fold exactly into the pos matrix (b==0 in
  practice; a general-b path adds ones-row matmuls + edge masks).
- attn head->channel expansion is done by SBUF->SBUF broadcast DMAs
  (source AP replicates each attn row 32x via a stride-0 dim), issued from
  whichever engine has slack - this removes all expand matmuls from PE.
- logits rows are ordered dl-major (row = dl*8 + head) so the expand DMA
  reads contiguous partitions.
- q/k/v are stored t-merged [128, 2, px] bf16 so each elementwise product
  handles both channel halves in one op (DVE 2x bf16 mode / Pool flat).
- PE is pre-warmed with dummy matmuls during the input-DMA head so real
  matmuls run at full clock; chunks are software-pipelined
  (logits(ci+1) on PE overlaps AV-products(ci) on DVE/Pool).
"""
import numpy as np

import concourse.mybir as mybir
import concourse.tile as tile
from concourse import bacc

F32 = mybir.dt.float32
F32R = mybir.dt.float32r
BF16 = mybir.dt.bfloat16
AF = mybir.ActivationFunctionType

B, CIN, H, W = 4, 256, 64, 64
QK = 256
OUT = 256
NH = 8
KW = 3
D = QK // NH          # 32
SCALE = float(D) ** (-0.25)
NCORES = 8

HOWN = H // 2         # 32 own rows per core
HS = HOWN + 2         # 34 rows incl halo
WP = W + 4            # 68 padded width (interior cols 2..65)
C0 = 2
PXU = HS * W          # 2176 projection pixels
OWNPX = HOWN * W      # 2048
NKK = KW * KW         # 9
NL = NH * NKK         # 72

CHUNK = 512           # 8 rows per attention chunk
NCHUNK = OWNPX // CHUNK
PXC = [448, 448, 448, 448, 384]           # proj px chunks (7,7,7,7,6 rows)
PXO = [0, 448, 896, 1344, 1792]


def _build_nc(has_bias: bool):
    nc = bacc.Bacc()

    xin = nc.declare_dram_parameter("x", [CIN, PXU], F32, isOutput=False)
    wtd = nc.declare_dram_parameter("wt", [CIN, 3 * QK], F32, isOutput=False)
    posd = nc.declare_dram_parameter("posm", [CIN, NL], BF16, isOutput=False)
    redd = nc.declare_dram_parameter("redm", [CIN, NKK * NL], BF16, isOutput=False)
    sum9d = nc.declare_dram_parameter("sum9", [NL, NH], BF16, isOutput=False)
    e8d = nc.declare_dram_parameter("e8", [NH, NL], BF16, isOutput=False)
    expd = nc.declare_dram_parameter("expm", [NL, 2 * NKK * 128], BF16, isOutput=False)
    identd = nc.declare_dram_parameter("ident", [128, 128], BF16, isOutput=False)
    if has_bias:
        biasd = nc.declare_dram_parameter("bias", [1, 3 * QK], F32, isOutput=False)
        edged = nc.declare_dram_parameter("edge", [128, 2], F32, isOutput=False)
    outd = nc.declare_dram_parameter("o", [OUT, OWNPX], F32, isOutput=True)

    with tile.TileContext(nc) as tc:
        with (
            tc.tile_pool(name="const", bufs=1) as constp,
            tc.tile_pool(name="data", bufs=1) as datap,
            tc.tile_pool(name="work", bufs=4) as workp,
            tc.tile_pool(name="psp", bufs=4, space="PSUM") as psp,   # proj+expand
            tc.tile_pool(name="psl", bufs=1, space="PSUM") as psl,   # logits
            tc.tile_pool(name="psz", bufs=1, space="PSUM") as psz,   # Z + bcast
            tc.tile_pool(name="pso", bufs=1, space="PSUM") as pso,   # AV out
        ):
            # ---- input DMAs: wt on SP/ACT (long poles), x quarters on all
            #      three DMA-capable engines, earliest chunks first ----
            x_t = [datap.tile([128, PXU], F32, tag=f"x{t}", name=f"x{t}")
                   for t in range(2)]
            wt_t = [datap.tile([128, 3 * QK], F32, tag=f"wt{t}", name=f"wt{t}")
                    for t in range(2)]
            # x segment grid aligned to the proj px chunks: q0 = 448 (whole
            # first px chunk), then 288-wide segments. Critical-path first:
            # x q0 halves and wt chunk0 each on their own engine.
            XSEG = [(0, 448), (448, 288), (736, 288), (1024, 288),
                    (1312, 288), (1600, 288), (1888, 288)]
            # ACT's queue starts with a 1.3us LoadActFuncSet, so critical
            # DMAs go on SP/Pool only (DMA data lands at busy_end + ~1.7us)
            nc.gpsimd.dma_start(x_t[0][:, 0:448], xin[0:128, 0:448])
            nc.sync.dma_start(x_t[1][:, 0:448], xin[128:256, 0:448])
            nc.gpsimd.dma_start(wt_t[1][:, 0:256], wtd[128:256, 0:256])
            nc.sync.dma_start(wt_t[0][:, 0:256], wtd[0:128, 0:256])
            for mi in range(1, 3):
                nc.sync.dma_start(wt_t[0][:, mi * 256:(mi + 1) * 256],
                                  wtd[0:128, mi * 256:(mi + 1) * 256])
                nc.scalar.dma_start(wt_t[1][:, mi * 256:(mi + 1) * 256],
                                    wtd[128:256, mi * 256:(mi + 1) * 256])
            XORD = [(1, 0, nc.gpsimd), (1, 1, nc.gpsimd),
                    (2, 0, nc.sync), (2, 1, nc.scalar),
                    (3, 0, nc.sync), (3, 1, nc.scalar),
                    (4, 0, nc.gpsimd), (4, 1, nc.gpsimd),
                    (5, 0, nc.sync), (5, 1, nc.scalar),
                    (6, 0, nc.gpsimd), (6, 1, nc.sync)]
            for qi, t, eng in XORD:
                o0, w = XSEG[qi]
                eng.dma_start(x_t[t][:, o0:o0 + w],
                              xin[t * 128:(t + 1) * 128, o0:o0 + w])
            if has_bias:
                bias_t = constp.tile([1, 3 * QK], F32, tag="bias", name="bias")
                nc.sync.dma_start(bias_t[:], biasd[:])
                edge_t = constp.tile([128, 2], F32, tag="edge", name="edge")
                nc.sync.dma_start(edge_t[:], edged[:])
                ones_t = constp.tile([1, max(PXC)], F32, tag="ones", name="ones")
                nc.gpsimd.memset(ones_t[:], 1.0)

            # ---- q/k/v storage: t-merged bf16; k/v width-padded with halo ----
            q_b = datap.tile([128, 2, PXU], BF16, tag="qb", name="qb")
            k_b = datap.tile([128, 2, HS * WP], BF16, tag="kb", name="kb")
            v_b = datap.tile([128, 2, HS * WP], BF16, tag="vb", name="vb")
            for tl in (k_b, v_b):
                fv = tl[:].bitcast(F32).rearrange("p t (r c) -> p t r c",
                                                  c=WP // 2)
                nc.gpsimd.memset(fv[:, :, :, 0:1], 0.0)
                nc.gpsimd.memset(fv[:, :, :, WP // 2 - 1:WP // 2], 0.0)

            # ---- projection: psum -> writebacks (plain casts; bias via
            #      ones-row matmul only when has_bias) ----
            # f32r operands must be rounded by a compute op (BIR verifier)
            x_rt = [datap.tile([128, PXU], F32R, tag=f"xr{t}", name=f"xr{t}")
                    for t in range(2)]
            wt_rt = [datap.tile([128, 3 * QK], F32R, tag=f"wtr{t}", name=f"wtr{t}")
                     for t in range(2)]
            # fine-grained rounding copies, critical-path first: wt chunk 0
            # and x q0/q1 before the later wt chunks
            def wt_copy(mi, t, eng):
                eng.tensor_copy(wt_rt[t][:, mi * 256:(mi + 1) * 256],
                                wt_t[t][:, mi * 256:(mi + 1) * 256])

            def x_copy(qi, t, eng):
                o0, w = XSEG[qi]
                eng.tensor_copy(x_rt[t][:, o0:o0 + w], x_t[t][:, o0:o0 + w])

            # first-matmul inputs split DVE/Pool so all four land ~3.1us
            x_copy(0, 0, nc.vector)
            x_copy(0, 1, nc.gpsimd)
            wt_copy(0, 0, nc.vector)
            wt_copy(0, 1, nc.gpsimd)
            x_copy(1, 0, nc.vector)
            x_copy(1, 1, nc.gpsimd)
            wt_copy(1, 0, nc.vector)
            wt_copy(1, 1, nc.gpsimd)
            wt_copy(2, 0, nc.gpsimd)
            wt_copy(2, 1, nc.vector)
            for qi in range(2, 7):
                for t in range(2):
                    eng = nc.vector if (qi + t) % 2 == 0 else nc.gpsimd
                    x_copy(qi, t, eng)
            x_r = [x_rt[t][:] for t in range(2)]
            wt_r = [wt_rt[t][:] for t in range(2)]

            # attention constants: emitted after the proj-critical copies,
            # issued from SP/ACT so Pool's queue stays clear
            pos_r = [constp.tile([128, NL], BF16, tag=f"pos{t}", name=f"pos{t}")
                     for t in range(2)]
            red_r = [constp.tile([128, NKK * NL], BF16, tag=f"red{t}", name=f"red{t}")
                     for t in range(2)]
            for t in range(2):
                nc.sync.dma_start(pos_r[t][:], posd[t * 128:(t + 1) * 128, :])
                nc.scalar.dma_start(red_r[t][:], redd[t * 128:(t + 1) * 128, :])
            sum9_r = constp.tile([NL, NH], BF16, tag="sum9", name="sum9")
            nc.sync.dma_start(sum9_r[:], sum9d[:])
            e8_r = constp.tile([NH, NL], BF16, tag="e8", name="e8")
            nc.sync.dma_start(e8_r[:], e8d[:])
            exp_r = constp.tile([NL, 2 * NKK * 128], BF16, tag="expm", name="expm")
            nc.scalar.dma_start(exp_r[:], expd[:])
            ident_r = constp.tile([128, 128], BF16, tag="ident", name="ident")
            nc.sync.dma_start(ident_r[:], identd[:])

            # gpsimd cannot access PSUM: writebacks on ACT/DVE only
            wb_engs = [nc.scalar, nc.scalar, nc.vector]

            def pad_view(tl, t, r0, nr, c0, cw=W):
                v = tl[:].rearrange("p t (r c) -> p t r c", c=WP)
                return v[:, t, r0:r0 + nr, c0:c0 + cw]

            wb_i = 0
            for ci in range(5):
                cw, co = PXC[ci], PXO[ci]
                r0, nr = co // W, cw // W
                for m in range(6):
                    grp, t = m // 2, m % 2
                    pp = psp.tile([128, CHUNK], F32, tag="pp", name="pp")
                    for kt in range(2):
                        nc.tensor.matmul(
                            pp[:, :cw],
                            wt_r[kt][:, m * 128:(m + 1) * 128],
                            x_r[kt][:, co:co + cw],
                            start=(kt == 0), stop=(not has_bias and kt == 1),
                            skip_group_check=True,
                        )
                    if has_bias:
                        nc.tensor.matmul(
                            pp[:, :cw], bias_t[:, m * 128:(m + 1) * 128],
                            ones_t[:, :cw],
                            start=False, stop=True, skip_group_check=True,
                        )
                    if grp == 0:
                        ov = q_b[:, t, co:co + cw].rearrange(
                            "p (r c) -> p r c", c=W)
                    else:
                        ov = pad_view(k_b if grp == 1 else v_b, t, r0, nr, C0)
                    eng = wb_engs[wb_i % len(wb_engs)]
                    wb_i += 1
                    if eng is nc.scalar:
                        nc.scalar.copy(ov, pp[:, :cw].rearrange(
                            "p (r c) -> p r c", c=W))
                    else:
                        eng.tensor_copy(ov, pp[:, :cw].rearrange(
                            "p (r c) -> p r c", c=W))

            if has_bias:
                # zero k/v halo rows that fall outside the image
                for tl in (k_b, v_b):
                    pv = tl[:].rearrange("p t (r c) -> p t r c", c=WP)
                    nc.gpsimd.tensor_scalar_mul(pv[:, :, 0, :], pv[:, :, 0, :],
                                                edge_t[:, 0:1])
                    nc.gpsimd.tensor_scalar_mul(pv[:, :, HS - 1, :],
                                                pv[:, :, HS - 1, :],
                                                edge_t[:, 1:2])

            # ---- attention chunks ----
            def qview(ci):
                return q_b[:].rearrange("p t (r c) -> p t r c", c=W)[
                    :, :, 1 + 8 * ci:9 + 8 * ci, :]

            def kv_view(tl, ci, di, dj):
                return tl[:].rearrange("p t (r c) -> p t r c", c=WP)[
                    :, :, 8 * ci + di:8 * ci + di + 8, dj + 1:dj + 1 + W]

            # product engine assignment per dl: reduce phase / AV phase
            RED_ENG = [0, 1, 0, 0, 1, 0, 0, 1, 0]   # 0=DVE (6), 1=Pool (3)
            # AV: 0 = DVE direct from psum; 1 = ACT cast to SBUF + Pool mul
            AV_ENG = [(0, 1), (1, 0), (0, 1), (1, 0), (0, 1), (1, 0),
                      (0, 1), (1, 0), (0, 0)]       # DVE 10, ACT+Pool 8

            def emit_logits(ci):
                prs = []
                for dl in range(NKK):
                    di, dj = dl // KW, dl % KW
                    pr = workp.tile([128, 2, CHUNK], BF16, tag="pr",
                                    name=f"pr{ci}_{dl}", bufs=6)
                    eng = nc.vector if RED_ENG[dl] == 0 else nc.gpsimd
                    eng.tensor_mul(
                        pr[:].rearrange("p t (r c) -> p t r c", c=W),
                        qview(ci), kv_view(k_b, ci, di, dj))
                    prs.append(pr)
                pl = psl.tile([NL, CHUNK], F32, tag="pl", name=f"pl{ci}", bufs=1)
                for t in range(2):
                    nc.tensor.matmul(pl[:], pos_r[t][:],
                                     q_b[:, t, 64 + CHUNK * ci:64 + CHUNK * (ci + 1)],
                                     start=(t == 0), stop=False,
                                     skip_group_check=True)
                for dl in range(NKK):
                    for t in range(2):
                        nc.tensor.matmul(
                            pl[:], red_r[t][:, dl * NL:(dl + 1) * NL],
                            prs[dl][:, t, :],
                            start=False, stop=(dl == NKK - 1 and t == 1),
                            skip_group_check=True)
                e_t = workp.tile([NL, CHUNK], BF16, tag="e", name=f"e{ci}", bufs=2)
                nc.scalar.activation(e_t[:], pl[:], AF.Exp)
                zz = psz.tile([NL, CHUNK], F32, tag="zz", name=f"zz{ci}", bufs=1)
                nc.tensor.matmul(zz[:][64:72], sum9_r[:], e_t[:],
                                 start=True, stop=True, skip_group_check=True)
                rz = workp.tile([NH, CHUNK], BF16, tag="rz", name=f"rz{ci}", bufs=2)
                with nc.allow_low_precision(reason="bf16 softmax denominators"):
                    nc.vector.reciprocal(rz[:], zz[:][64:72])
                nc.tensor.matmul(zz[:][0:NL], e8_r[:], rz[:],
                                 start=True, stop=True, skip_group_check=True)
                attn = workp.tile([NL, CHUNK], BF16, tag="attn",
                                  name=f"attn{ci}", bufs=2)
                nc.vector.tensor_mul(attn[:], e_t[:], zz[:][0:NL])
                return attn

            def emit_av(ci, attn):
                # per dl: 2 expand matmuls (psum, pp slots) -> 2 products
                # -> 1 flat ident matmul accumulating both halves into po
                po = pso.tile([128, 2, CHUNK], F32, tag="po", name=f"po{ci}",
                              bufs=1)
                p2s = [None] * NKK

                def emit_exp_prod(dl):
                    di, dj = dl // KW, dl % KW
                    p2 = workp.tile([128, 2, CHUNK], BF16, tag="p2",
                                    name=f"p2{ci}_{dl}", bufs=5)
                    for t in range(2):
                        pe = psp.tile([128, CHUNK], F32, tag="pp",
                                      name=f"ax{ci}_{dl}_{t}")
                        nc.tensor.matmul(
                            pe[:], exp_r[:, (dl * 2 + t) * 128:(dl * 2 + t + 1) * 128],
                            attn[:], start=True, stop=True,
                            skip_group_check=True)
                        if AV_ENG[dl][t] == 0:
                            # DVE multiplies straight from psum
                            nc.vector.tensor_mul(
                                p2[:, t, :].rearrange("p (r c) -> p r c", c=W),
                                pe[:].rearrange("p (r c) -> p r c", c=W),
                                kv_view(v_b, ci, di, dj)[:, t])
                        else:
                            # gpsimd can't read psum: ACT casts, Pool multiplies
                            axs = workp.tile([128, CHUNK], BF16, tag="axs",
                                             name=f"axs{ci}_{dl}_{t}", bufs=4)
                            nc.scalar.copy(axs[:], pe[:])
                            nc.gpsimd.tensor_mul(
                                p2[:, t, :].rearrange("p (r c) -> p r c", c=W),
                                axs[:].rearrange("p (r c) -> p r c", c=W),
                                kv_view(v_b, ci, di, dj)[:, t])
                    p2s[dl] = p2

                def emit_ident(dl, t):
                    nc.tensor.matmul(
                        po[:, t, :], ident_r[:], p2s[dl][:, t, :],
                        start=(dl == 0), stop=(dl == NKK - 1),
                        skip_group_check=True)

                # t0 chain runs one dl ahead of t1 so po[t0] closes first
                # and its drain overlaps the final t1 idents
                emit_exp_prod(0)
                emit_exp_prod(1)
                emit_ident(0, 0)
                for dl in range(2, NKK):
                    emit_exp_prod(dl)
                    emit_ident(dl - 1, 0)
                    emit_ident(dl - 2, 1)
                emit_ident(NKK - 1, 0)
                ob = workp.tile([128, 2, CHUNK], F32, tag="ob",
                                name=f"ob{ci}", bufs=2)
                ovw = outd[:].rearrange("(t c) px -> c t px", t=2)
                nc.scalar.copy(ob[:, 0, :], po[:, 0, :])
                nc.sync.dma_start(ovw[:, 0, ci * CHUNK:(ci + 1) * CHUNK],
                                  ob[:, 0, :])
                emit_ident(NKK - 2, 1)
                emit_ident(NKK - 1, 1)
                if ci == NCHUNK - 1:
                    # tail: drain t1 in two half-px pieces so the first DMA
                    # issues while the second half is still being copied
                    hc = CHUNK // 2
                    for h in range(2):
                        nc.scalar.copy(ob[:, 1, h * hc:(h + 1) * hc],
                                       po[:, 1, h * hc:(h + 1) * hc])
                        nc.sync.dma_start(
                            ovw[:, 1, ci * CHUNK + h * hc:ci * CHUNK + (h + 1) * hc],
                            ob[:, 1, h * hc:(h + 1) * hc])
                else:
                    nc.scalar.copy(ob[:, 1, :], po[:, 1, :])
                    nc.sync.dma_start(ovw[:, 1, ci * CHUNK:(ci + 1) * CHUNK],
                                      ob[:, 1, :])

            # software pipeline: logits(ci+1) on PE ahead of AV(ci)
            attn_prev = emit_logits(0)
            for ci in range(1, NCHUNK):
                attn_c = emit_logits(ci)
                emit_av(ci - 1, attn_prev)
                attn_prev = attn_c
            emit_av(NCHUNK - 1, attn_prev)

    nc.finalize()
    return nc


_CACHE = {}


def _host_consts(w_proj, b_proj, pos_feats):
    wT = np.ascontiguousarray(w_proj.T).astype(np.float32).copy()   # [256, 768]
    wT[:, :2 * QK] *= SCALE

    import ml_dtypes
    bf = ml_dtypes.bfloat16

    heads = np.arange(CIN) // D
    posm = np.zeros((CIN, NL), np.float32)
    for g in range(CIN):
        n = heads[g]
        for dl in range(NKK):
            posm[g, dl * NH + n] = pos_feats[g, dl]

    redm = np.zeros((CIN, NKK * NL), np.float32)
    for t in range(2):
        for c in range(128):
            n = heads[t * 128 + c]
            for dl in range(NKK):
                redm[t * 128 + c, dl * NL + dl * NH + n] = 1.0
    # NOTE: redm rows are global channels; tile t uses rows t*128..t*128+127

    sum9 = np.zeros((NL, NH), np.float32)
    e8 = np.zeros((NH, NL), np.float32)
    for n in range(NH):
        for dl in range(NKK):
            sum9[dl * NH + n, n] = 1.0
            e8[n, dl * NH + n] = 1.0

    expm = np.zeros((NL, 2 * NKK * 128), np.float32)
    for dl in range(NKK):
        for t in range(2):
            for c in range(128):
                expm[dl * NH + t * 4 + c // 32, (dl * 2 + t) * 128 + c] = 1.0

    ident = np.eye(128, dtype=np.float32)

    b = np.asarray(b_proj, np.float32).copy()
    b[:2 * QK] *= SCALE
    bias = np.ascontiguousarray(b.reshape(1, 3 * QK))

    return (wT, posm.astype(bf), redm.astype(bf), sum9.astype(bf),
            e8.astype(bf), expm.astype(bf), ident.astype(bf), bias)


def make_in_maps(x, w_proj, b_proj, pos_feats):
    x = np.asarray(x, np.float32)
    has_bias = bool(np.any(np.asarray(b_proj)))
    wT, posm, redm, sum9, e8, expm, ident, bias = _host_consts(
        np.asarray(w_proj, np.float32),
        np.asarray(b_proj, np.float32),
        np.asarray(pos_feats, np.float32),
    )
    in_maps = []
    for s in range(NCORES):
        b_i, half = s // 2, s % 2
        xs = np.zeros((CIN, HS, W), np.float32)
        h0 = half * HOWN - 1
        lo, hi = max(h0, 0), min(h0 + HS, H)
        xs[:, lo - h0:hi - h0, :] = x[b_i, :, lo:hi, :]
        m = {
            "x": np.ascontiguousarray(xs.reshape(CIN, PXU)),
            "wt": wT, "posm": posm, "redm": redm,
            "sum9": sum9, "e8": e8, "expm": expm, "ident": ident,
        }
        if has_bias:
            edge = np.ones((128, 2), np.float32)
            if half == 0:
                edge[:, 0] = 0.0
            if half == 1:
                edge[:, 1] = 0.0
            m["bias"] = bias
            m["edge"] = edge
        in_maps.append(m)
    return in_maps, has_bias


def kernel(x, w_proj, b_proj, pos_feats):
    from concourse.bass_utils import run_bass_kernel_spmd

    in_maps, has_bias = make_in_maps(x, w_proj, b_proj, pos_feats)
    key = ("nc", has_bias)
    if key not in _CACHE:
        _CACHE[key] = _build_nc(has_bias)
        _CACHE["nc"] = _CACHE[key]
    nc = _CACHE[key]
    res = run_bass_kernel_spmd(nc, in_maps, list(range(NCORES)))
    out = np.zeros((B, OUT, H, W), np.float32)
    for s in range(NCORES):
        b_i, half = s // 2, s % 2
        out[b_i, :, half * HOWN:(half + 1) * HOWN, :] = (
            res.results[s]["o"].reshape(OUT, HOWN, W)
        )
    return out
